# revision 7
# baseline (speedup 1.0000x reference)
"""Trainium2 Bass kernel for nn_BipolarCellNetwork.

Sharding: data-parallel over batch — 64 stimulus rows split as 8 rows per
NeuronCore, all parameters replicated (per spec sharding hint). Each core runs
an identical program on its slice; no collectives.

Per core:
  * 52-tap causal conv of x computed in bulk 256-col chunks on the tensor
    engine (lhsT = bc_kernels.T, rhs = overlapping sliding-window DMA of
    zero-padded x).
  * LNR branch: given p = sigmoid(slope*(xf-off)), the pool recurrence is
    linear: pool_t = a_t*pool_{t-1} + b_t with a=(1-rho)(1-p), b=rho(1-p)
    -> one tensor_tensor_scan per 256-chunk. Runs on the otherwise-idle Pool
    engine + ACT, interleaved with the BCN loop so it hides under it.
  * BCN branch: genuinely sequential recurrence over T steps (lag-1 feedback
    through two sigmoids), latency-bound on the cross-engine chain. The AC
    temporal kernel on the uniform time grid is exactly A*r^i + B*q^i
    (difference of exponentials), so the 52-tap history convolution reduces
    to two geometric states with a t-53 tail correction:
        psA  = Wg @ rel_{t-1} - Wgk @ rel_{t-53}     (PE -> PSUM)
        S    = g*S + psA                             (DVE scalar_tensor_tensor)
        drive= S[0:45] + S[64:109]                   (DVE; S2 parked at
                                                      partition 64 to satisfy
                                                      the base-partition rule)
        ac   = sigmoid(acs*drive + acb)              (ACT, direct to staging)
        z    = xf_t + AW @ ac                        (PE, identity-mm + mm)
        p    = sigmoid(sls*z + slb)                  (ACT)
        rel  = p * pool2                             (DVE, direct to staging)
        pool2= (1-rho)*(pool2-rel) + rho             (DVE, off critical path)
    rel/fb/ac are staged in SBUF rings (256 steps x 2 buffers), transposed
    128-col blocks via PE matmuls (out_scale folded into a diagonal rhs) and
    DMA'd to HBM as contiguous blocks.
"""

import numpy as np

NBC, NAC, KF, RHO = 14, 45, 52, 0.05
K = 52
B_FULL, T_FULL = 64, 4096
NCORES = 8
BL = B_FULL // NCORES      # 8 batch rows per core
CH = 256                   # loop chunk / staging ring size
PAD = 64                   # zeros prepended to x (covers conv warmup)
SP = 109                   # S-state partitions: S1 at 0..44, S2 at 64..108


# ---------------------------------------------------------------- host prep
def _host_consts(bc_kernels, log_sigmoid_slope, sigmoid_offset, log_bc_ac_weight,
                 log_ac_tau_rise, log_ac_tau_decay, log_ac_sigmoid_slope,
                 ac_sigmoid_offset, log_ac_bc_weight, out_scale):
    f32 = np.float32
    kt = np.arange(0.8, 0, -1.0 / 64, dtype=f32)          # (52,)
    td = np.exp(np.asarray(log_ac_tau_decay, f32))
    tr = np.exp(np.asarray(log_ac_tau_rise, f32))
    k_raw = np.exp(-kt[None, :] / td[:, None]) - np.exp(
        -(td + tr)[:, None] / (td * tr)[:, None] * kt[None, :])
    norm = np.linalg.norm(k_raw.astype(f32), axis=1)       # (45,)
    alpha = (td + tr) / (td * tr)
    r = np.exp(-1.0 / (64.0 * td)).astype(f32)
    q = np.exp(-alpha / 64.0).astype(f32)
    A = (np.exp(1.0 / (80.0 * td)) / norm).astype(f32)
    Bc = (-np.exp(alpha / 80.0) / norm).astype(f32)

    W = np.exp(np.asarray(log_bc_ac_weight, f32))          # (45, 14)
    wg1 = (r * A)[:, None] * W                             # (45, 14)
    wgk1 = -((r ** 53) * A)[:, None] * W
    wgs = (r * A + q * Bc)[:, None] * W
    wgks = -((r ** 53) * A + (q ** 53) * Bc)[:, None] * W

    slope = np.exp(np.asarray(log_sigmoid_slope, f32))
    ac_slope = np.exp(np.asarray(log_ac_sigmoid_slope, f32))
    off = np.asarray(sigmoid_offset, f32)
    ac_off = np.asarray(ac_sigmoid_offset, f32)
    osc = np.asarray(out_scale, f32)
    cc = np.ascontiguousarray
    return {
        "kft": cc(np.asarray(bc_kernels, f32).T),           # (52, 14)
        "wg1": cc(wg1.T.astype(f32)),                       # (14, 45)
        "wgk1": cc(wgk1.T.astype(f32)),                     # (14, 45)
        "wgs": cc(wgs.T.astype(f32)),                       # (14, 45)
        "wgks": cc(wgks.T.astype(f32)),                     # (14, 45)
        "drq": cc(np.diag((r - q).astype(f32))),            # (45, 45)
        "aw": cc((-np.exp(np.asarray(log_ac_bc_weight, f32))).T),  # (45, 14)
        "rvec": cc(r[:, None]),                             # (45, 1)
        "qvec": cc(q[:, None]),                             # (45, 1)
        "acs": cc(ac_slope[:, None]),                       # (45, 1)
        "acb": cc((-ac_slope * ac_off)[:, None]),           # (45, 1)
        "sls": cc(slope[:, None]),                          # (14, 1)
        "slb": cc((-slope * off)[:, None]),                 # (14, 1)
        "dos": cc(np.diag(osc)),                            # (14, 14)
        "i14": np.eye(NBC, dtype=f32),
        "rhov": np.full((NBC, 1), RHO, f32),
        "i45": np.eye(NAC, dtype=f32),
    }


_CSHAPE = {"kft": [K, NBC], "wg1": [NBC, NAC], "wgk1": [NBC, NAC],
           "wgs": [NBC, NAC], "wgks": [NBC, NAC], "drq": [NAC, NAC],
           "aw": [NAC, NBC], "rvec": [NAC, 1], "qvec": [NAC, 1],
           "acs": [NAC, 1], "acb": [NAC, 1], "sls": [NBC, 1],
           "slb": [NBC, 1], "dos": [NBC, NBC], "i14": [NBC, NBC], "i45": [NAC, NAC],
           "rhov": [NBC, 1]}


# ---------------------------------------------------------------- device prog
def build_nc(T):
    import concourse.bass as bass
    import concourse.tile as tile
    from concourse import bacc, mybir

    f32 = mybir.dt.float32
    AF = mybir.ActivationFunctionType
    ALU = mybir.AluOpType
    NCH = T // CH
    assert T % CH == 0 and NCH >= 2

    nc = bacc.Bacc("TRN2", debug=False)

    xp_d = nc.dram_tensor("xp", [BL, T + PAD], f32, kind="ExternalInput")
    cd = {k: nc.dram_tensor(k, v, f32, kind="ExternalInput")
          for k, v in _CSHAPE.items()}
    y_d = nc.dram_tensor("y", [BL, T, NBC], f32, kind="ExternalOutput")
    fb_d = nc.dram_tensor("fb", [BL, T, NBC], f32, kind="ExternalOutput")
    ac_d = nc.dram_tensor("ac", [BL, T, NAC], f32, kind="ExternalOutput")
    yl_d = nc.dram_tensor("ylnr", [BL, T, NBC], f32, kind="ExternalOutput")

    with tile.TileContext(nc) as tc:
        with (tc.tile_pool(name="const", bufs=1) as cpool,
              tc.tile_pool(name="state", bufs=1) as spool,
              tc.tile_pool(name="work", bufs=3) as wpool,
              tc.tile_pool(name="stage", bufs=4) as stpool,
              tc.tile_pool(name="ppA1", bufs=1, space="PSUM") as ppA1,
              tc.tile_pool(name="ppAD", bufs=1, space="PSUM") as ppAD,
              tc.tile_pool(name="ppB", bufs=1, space="PSUM") as ppB,
              tc.tile_pool(name="ppB2", bufs=1, space="PSUM") as ppB2,
              tc.tile_pool(name="ppC", bufs=1, space="PSUM") as ppC,
              tc.tile_pool(name="ppT", bufs=3, space="PSUM") as ppT):

            cs = {}
            for k, shp in _CSHAPE.items():
                cs[k] = cpool.tile(shp, f32, tag=f"c_{k}", name=f"c_{k}")
                nc.gpsimd.dma_start(cs[k][:], cd[k].ap()[:, :])

            s1 = spool.tile([NAC, BL], f32, tag="s1", name="s1")
            D = spool.tile([NAC, BL], f32, tag="D", name="D")
            pool2 = spool.tile([NBC, BL], f32, tag="pool2", name="pool2")
            stg_rel = [spool.tile([NBC, BL * CH], f32, tag=f"stgrel{i}", name=f"stgrel{i}") for i in range(2)]
            stg_fb = [spool.tile([NBC, BL * CH], f32, tag=f"stgfb{i}", name=f"stgfb{i}") for i in range(2)]
            stg_ac = [spool.tile([NAC, BL * CH], f32, tag=f"stgac{i}", name=f"stgac{i}") for i in range(2)]
            xfc = [spool.tile([NBC, BL * CH], f32, tag=f"xfc{i}", name=f"xfc{i}") for i in range(3)]
            lcar = [spool.tile([NBC, 1], f32, tag=f"lcar{n}", name=f"lcar{n}") for n in range(BL)]

            nc.gpsimd.memset(s1[:], 0.0)
            nc.gpsimd.memset(D[:], 0.0)
            nc.gpsimd.memset(pool2[:], 1.0)
            for i in range(2):
                nc.gpsimd.memset(stg_rel[i][:], 0.0)
            for n in range(BL):
                nc.gpsimd.memset(lcar[n][:], 1.0)

            def stg_col(stg, k):
                """[P, BL] strided column view (cols n*CH + k)."""
                return stg[:].rearrange("p (n k) -> p n k", k=CH)[:, :, k]

            def rel_slice(t):
                if t < 0:
                    return stg_col(stg_rel[1], t % CH)
                return stg_col(stg_rel[(t // CH) % 2], t % CH)

            # ---- conv + LNR unit for (n, chunk c) --------------------------
            def emit_conv_lnr(n, c):
                xwin = wpool.tile([K, CH], f32, tag="xwin", name="xwin")
                win = bass.AP(xp_d, n * (T + PAD) + c * CH + 13, [[1, K], [1, CH]])
                nc.gpsimd.dma_start(xwin[:], win)

                psC = ppC.tile([NBC, CH], f32, tag="psC", name="psC")
                nc.tensor.matmul(psC[:, :], cs["kft"][:], xwin[:], start=True, stop=True)
                nc.vector.tensor_copy(xfc[c % 3][:, n * CH:(n + 1) * CH], psC[:, :])

                pL = wpool.tile([NBC, CH], f32, tag="pL", name="pL")
                nc.scalar.activation(pL[:], psC[:, :], AF.Sigmoid,
                                     bias=cs["slb"][:], scale=cs["sls"][:])
                aL = wpool.tile([NBC, CH], f32, tag="aL", name="aL")
                bL = wpool.tile([NBC, CH], f32, tag="bL", name="bL")
                nc.gpsimd.tensor_scalar(aL[:], pL[:], -(1.0 - RHO), (1.0 - RHO),
                                        op0=ALU.mult, op1=ALU.add)
                nc.gpsimd.tensor_scalar(bL[:], pL[:], -RHO, RHO,
                                        op0=ALU.mult, op1=ALU.add)
                scanT = wpool.tile([NBC, CH + 1], f32, tag="scanT", name="scanT")
                nc.gpsimd.tensor_copy(scanT[:, 0:1], lcar[n][:])
                nc.vector.tensor_tensor_scan(scanT[:, 1:CH + 1], aL[:], bL[:],
                                             scanT[:, 0:1], op0=ALU.mult, op1=ALU.add)
                nc.gpsimd.tensor_copy(lcar[n][:], scanT[:, CH:CH + 1])
                p2L = wpool.tile([NBC, CH], f32, tag="p2L", name="p2L")
                nc.scalar.activation(p2L[:], scanT[:, 0:CH], AF.Identity,
                                     bias=cs["rhov"][:], scale=1.0 - RHO)
                relL = wpool.tile([NBC, CH], f32, tag="relL", name="relL")
                nc.gpsimd.tensor_mul(relL[:], pL[:], p2L[:])
                for h in range(CH // 128):
                    psT = ppT.tile([128, NAC], f32, tag="psT", name="psT")
                    nc.tensor.matmul(psT[:, 0:NBC], relL[:, 128 * h:128 * (h + 1)],
                                     cs["dos"][:], start=True, stop=True)
                    sbT = stpool.tile([128, NAC], f32, tag="sbT", name="sbT")
                    nc.scalar.copy(sbT[:, 0:NBC], psT[:, 0:NBC])
                    nc.gpsimd.dma_start(
                        yl_d.ap()[n, c * CH + 128 * h: c * CH + 128 * (h + 1), :],
                        sbT[:, 0:NBC])

            # ---- output flush for half-chunk (c, h) ------------------------
            def emit_flush(c, h):
                buf = c % 2
                t0 = c * CH + 128 * h
                for n in range(BL):
                    sl = slice(n * CH + 128 * h, n * CH + 128 * (h + 1))
                    for stg, rhs, width, dram in (
                            (stg_rel[buf], cs["dos"], NBC, y_d),
                            (stg_fb[buf], cs["i14"], NBC, fb_d),
                            (stg_ac[buf], cs["i45"], NAC, ac_d)):
                        psT = ppT.tile([128, NAC], f32, tag="psT", name="psT")
                        nc.tensor.matmul(psT[:, 0:width], stg[:, sl], rhs[:],
                                         start=True, stop=True)
                        sbT = stpool.tile([128, NAC], f32, tag="sbT", name="sbT")
                        if n % 2 == 0:
                            nc.scalar.copy(sbT[:, 0:width], psT[:, 0:width])
                        else:
                            nc.vector.tensor_copy(sbT[:, 0:width], psT[:, 0:width])
                        nc.gpsimd.dma_start(dram.ap()[n, t0:t0 + 128, :],
                                            sbT[:, 0:width])

            # ---- prologue: conv/LNR for chunks 0 and 1
            for c in range(min(2, NCH)):
                for n in range(BL):
                    emit_conv_lnr(n, c)

            # ---- main loop
            for t in range(T):
                c, k = divmod(t, CH)
                buf = c % 2

                psAD = ppAD.tile([NAC, BL], f32, tag="psAD", name="psAD")
                nc.tensor.matmul(psAD[:, :], cs["drq"][:], s1[:],
                                 start=True, stop=False)
                nc.tensor.matmul(psAD[:, :], cs["wgs"][:], rel_slice(t - 1),
                                 start=False, stop=False)
                nc.tensor.matmul(psAD[:, :], cs["wgks"][:], rel_slice(t - 53),
                                 start=False, stop=True)
                psA1 = ppA1.tile([NAC, BL], f32, tag="psA1", name="psA1")
                nc.tensor.matmul(psA1[:, :], cs["wg1"][:], rel_slice(t - 1),
                                 start=True, stop=False)
                nc.tensor.matmul(psA1[:, :], cs["wgk1"][:], rel_slice(t - 53),
                                 start=False, stop=True)
                psB = ppB.tile([NBC, BL], f32, tag="psB", name="psB")
                nc.tensor.matmul(psB[:, :], cs["i14"][:], stg_col(xfc[c % 3], k),
                                 start=True, stop=False)

                nc.vector.scalar_tensor_tensor(D[:], D[:], cs["qvec"][:], psAD[:, :],
                                               op0=ALU.mult, op1=ALU.add)
                nc.vector.scalar_tensor_tensor(s1[:], s1[:], cs["rvec"][:], psA1[:, :],
                                               op0=ALU.mult, op1=ALU.add)

                ac_sl = stg_col(stg_ac[buf], k)
                nc.scalar.activation(ac_sl, D[:], AF.Sigmoid,
                                     bias=cs["acb"][:], scale=cs["acs"][:])

                nc.tensor.matmul(psB[:, :], cs["aw"][:], ac_sl, start=False, stop=True)
                psB2 = ppB2.tile([NBC, BL], f32, tag="psB2", name="psB2")
                nc.tensor.matmul(psB2[:, :], cs["aw"][:], ac_sl, start=True, stop=True)

                p = wpool.tile([NBC, BL], f32, tag="p", name="p")
                nc.scalar.activation(p[:], psB[:, :], AF.Sigmoid,
                                     bias=cs["slb"][:], scale=cs["sls"][:])

                rel_sl = stg_col(stg_rel[buf], k)
                nc.vector.tensor_mul(rel_sl, p[:], pool2[:])

                tmp = wpool.tile([NBC, BL], f32, tag="tmp", name="tmp")
                nc.vector.tensor_sub(tmp[:], pool2[:], rel_sl)
                nc.vector.tensor_scalar(pool2[:], tmp[:], 1.0 - RHO, RHO,
                                        op0=ALU.mult, op1=ALU.add)

                nc.scalar.copy(stg_col(stg_fb[buf], k), psB2[:, :])

                if k % 32 == 16 and c + 2 < NCH:
                    emit_conv_lnr(k // 32, c + 2)
                if k == 127:
                    emit_flush(c, 0)
                elif k == CH - 1:
                    emit_flush(c, 1)

    nc.compile()
    return nc


_NC_CACHE = {}


def _get_nc(T):
    if T not in _NC_CACHE:
        _NC_CACHE[T] = build_nc(T)
    return _NC_CACHE[T]


def run_cores(x, consts, T, nc=None, **spmd_kwargs):
    """x: (64, T) full input. Returns dict of full-batch outputs (+ perf)."""
    from concourse.bass_utils import run_bass_kernel_spmd
    if nc is None:
        nc = _get_nc(T)
    x = np.asarray(x, np.float32)
    xpad = np.concatenate([np.zeros((x.shape[0], PAD), np.float32), x], axis=1)
    in_maps = []
    for i in range(NCORES):
        m = dict(consts)
        m["xp"] = np.ascontiguousarray(xpad[i * BL:(i + 1) * BL])
        in_maps.append(m)
    res = run_bass_kernel_spmd(nc, in_maps, core_ids=list(range(NCORES)),
                               **spmd_kwargs)
    outs = {}
    for name in ("y", "fb", "ac", "ylnr"):
        outs[name] = np.concatenate([res.results[i][name] for i in range(NCORES)],
                                    axis=0)
    outs["_res"] = res
    return outs


def kernel(x, bc_kernels, log_sigmoid_slope, sigmoid_offset, log_bc_ac_weight,
           log_ac_tau_rise, log_ac_tau_decay, log_ac_sigmoid_slope,
           ac_sigmoid_offset, log_ac_bc_weight, out_scale):
    x = np.asarray(x, np.float32)
    consts = _host_consts(bc_kernels, log_sigmoid_slope, sigmoid_offset,
                          log_bc_ac_weight, log_ac_tau_rise, log_ac_tau_decay,
                          log_ac_sigmoid_slope, ac_sigmoid_offset,
                          log_ac_bc_weight, out_scale)
    outs = run_cores(x, consts, x.shape[1])
    return outs["y"], outs["fb"], outs["ac"], outs["ylnr"]
